# revision 16
# baseline (speedup 1.0000x reference)
"""Trainium2 Bass kernel for nn_Attention_12146167513140.

Distributed dense attention over 8 NeuronCores; core c -> (batch c//4,
head-pair c%4).

Mask-sparsity restructure: softmax is shift-invariant, so mask1 only
matters through the product mask1*mask2 -- for an invalid (mask1=0) query
the row mask is constant and drops out, i.e. that row attends to ALL
keys unmasked; for a valid query only the valid (mask2=1) keys
contribute.  Host-side we permute queries invalid-first and keys
valid-first, then run two dense passes per core:
  pass A: q rows [0, CAQ*128)  x all 24 key chunks      (no mask)
  pass B: q rows [BSTART,3072) x BV full-valid chunks + NB boundary
          chunks (boundary = copy of chunks BV.. with an additive
          rank-1 mask row killing the invalid tail)
This cuts score/exp/PV work to ~75% and removes the mask row from the
main QK contraction.

Engine choreography per (block, key chunk): QK matmuls (PE) -> exp on
EITHER ScalarE (exact, fused *SCALE) or DVE (1-op Schraudolph: bf16
bit-pattern = int16(x*SCALE*128*log2e + (16256-C)); ~2% rel err) ->
PV matmuls accumulate [d|Z] x q in PSUM.  The softmax denominator Z
rides as a ones-column in PV; normalization happens on HOST after the
output projection (out = op0/Z0 + op1/Z1), so no on-device transposes
or reciprocals of Z are needed.  RMS-norm stats run on GpSimd (square/
rsqrt/scale) off a single batched PSUM->SBUF copy; projections + PE
transposes fill pipeline bubbles of the attend stream to keep the PE
p-state at max clock.
"""

import contextlib
import ctypes
import sys
import types

import numpy as np
import ml_dtypes

import concourse.bacc as bacc
import concourse.mybir as mybir
from concourse import bass_utils
from concourse.tile import TileContext
from concourse.alu_op_type import AluOpType
from concourse.mybir import ActivationFunctionType as AF


def _ensure_trace_support():
    """The container's antenv package lacks axon_hooks; bass_utils
    imports it when tracing is requested (e.g. via BASS_TRACE).  Install
    a functional shim so a traced run works instead of crashing, and
    make the artifact upload a no-op (no bucket access here)."""
    try:
        import antenv.axon_hooks  # noqa: F401
        return
    except ImportError:
        pass
    mod = types.ModuleType("antenv.axon_hooks")
    mod._hook = None
    mod.set_axon_ntff_profile_hook = lambda h: setattr(mod, "_hook", h)
    mod.get_axon_ntff_profile_hook = lambda: mod._hook
    try:
        import antenv
        sys.modules["antenv.axon_hooks"] = mod
        antenv.axon_hooks = mod
    except ImportError:
        sys.modules["antenv.axon_hooks"] = mod

    def _ntff_hook(so_path):
        try:
            lib = ctypes.CDLL(so_path)
        except OSError:
            return None
        if not hasattr(lib, "axon_start_nrt_profile"):
            return None
        lib.axon_start_nrt_profile.argtypes = [ctypes.POINTER(ctypes.c_int64),
                                               ctypes.c_size_t]
        lib.axon_start_nrt_profile.restype = ctypes.c_int64
        lib.axon_stop_nrt_profile.argtypes = [ctypes.c_char_p]
        lib.axon_stop_nrt_profile.restype = ctypes.c_int64

        @contextlib.contextmanager
        def _hook(output_dir, device_ids):
            import jax
            jax.devices()
            if device_ids:
                ids = (ctypes.c_int64 * len(device_ids))(*device_ids)
                rc = lib.axon_start_nrt_profile(ids, len(device_ids))
            else:
                rc = lib.axon_start_nrt_profile(None, 0)
            if rc != 0:
                raise RuntimeError(f"axon_start_nrt_profile rc={rc}")
            try:
                yield
            finally:
                lib.axon_stop_nrt_profile(str(output_dir).encode())

        return _hook

    mod.set_axon_ntff_profile_hook(_ntff_hook("/opt/axon/libaxon_pjrt.so"))

    _orig_upload = bass_utils.upload_artifacts

    def _safe_upload(tmpdir):
        try:
            return _orig_upload(tmpdir)
        except Exception:
            return tmpdir

    bass_utils.upload_artifacts = _safe_upload


_ensure_trace_support()

AX = mybir.AxisListType
I16 = mybir.dt.int16
I32 = mybir.dt.int32
BF = mybir.dt.bfloat16
F32 = mybir.dt.float32
bf16 = ml_dtypes.bfloat16

B, N1, N2 = 2, 3072, 3072
C_S, H, D = 256, 8, 32
INF = 100000.0
EPS = 1e-8
SCALE = float(np.sqrt(1.0 / (3 * D)))

NCORES = 8
HPC = 2              # heads per core
KCH = N2 // 128      # 24 real key chunks
VW = D + 1           # 33: v columns + ones column for Z

# Schraudolph exp in bf16 bit space: bits = x*SCALE*128*log2e + (16256 - C)
EXP_C = 5.77
EXP_A = SCALE * float(np.log2(np.e)) * 128.0
EXP_B = 127.0 * 128.0 - EXP_C

_cache = {}


def _build(cfg):
    CAQ, CBQ, BSTART, BV, NB, use_g2 = cfg
    CQ = CAQ + CBQ                 # total q chunks
    NQT = CQ * 128                 # q extent (gathered-virtual coordinate)
    NKC = KCH + NB                 # kv chunk sections incl boundary
    NKT = NKC * 128

    # q chunk -> source column in gathered s1T
    def qsrc(c):
        return c * 128 if c < CAQ else BSTART + (c - CAQ) * 128

    # attend blocks: (vq0, width, kc list, tag)
    blocks = []
    off, rem = 0, CAQ * 128
    while rem > 0:
        w = min(512, rem)
        blocks.append((off, w, list(range(KCH)), 'A'))
        off += w
        rem -= w
    bkcs = list(range(BV)) + [KCH + i for i in range(NB)]
    rem = CBQ * 128
    while rem > 0:
        w = min(512, rem)
        blocks.append((off, w, bkcs, 'B'))
        off += w
        rem -= w

    # exp engine: boundary chunks always ACT; elsewhere mix in DVE
    def exp_eng(bi, j, kc):
        if kc >= KCH:
            return 'a'
        if bi == 0:
            return 'v' if j % 4 == 2 else 'a'
        return 'v' if j % 5 in (1, 3) else 'a'

    nc = bacc.Bacc("TRN2", target_bir_lowering=False, debug=False,
                   num_devices=NCORES)

    s1T_d = nc.dram_tensor("s1T", [C_S, N1], BF, kind="ExternalInput")
    s2T_d = nc.dram_tensor("s2T", [C_S, N2], BF, kind="ExternalInput")
    wq_d = nc.dram_tensor("wq", [C_S, HPC * D], BF, kind="ExternalInput")
    wkv_d = nc.dram_tensor("wkv", [C_S, HPC * 2 * D], BF, kind="ExternalInput")
    wout_d = nc.dram_tensor("wout", [HPC * D, C_S], BF, kind="ExternalInput")
    id_d = nc.dram_tensor("ident", [128, 128], BF, kind="ExternalInput")
    kbnd_d = nc.dram_tensor("kbnd", [1, max(1, NB) * 128], BF,
                            kind="ExternalInput")
    if use_g2:
        g2_d = nc.dram_tensor("g2", [128, HPC * D], BF, kind="ExternalInput")
    op0_d = nc.dram_tensor("op0", [NQT, C_S], F32, kind="ExternalOutput")
    op1_d = nc.dram_tensor("op1", [NQT, C_S], F32, kind="ExternalOutput")
    z_d = nc.dram_tensor("z", [HPC, NQT], BF, kind="ExternalOutput")

    with TileContext(nc) as tc:
        with (
            tc.tile_pool(name="const", bufs=1) as cpool,
            tc.tile_pool(name="kcpp", bufs=3) as kcpp,
            tc.tile_pool(name="sqp", bufs=3) as sqp,
            tc.tile_pool(name="prep", bufs=6) as prep,
            tc.tile_pool(name="nrm", bufs=4) as nrm,
            tc.tile_pool(name="expp", bufs=5) as expp,
            tc.tile_pool(name="osp", bufs=4) as osp,
            tc.tile_pool(name="psSC", bufs=2, space="PSUM") as psSC,
            tc.tile_pool(name="psOT", bufs=2, space="PSUM") as psOT,
            tc.tile_pool(name="psX", bufs=2, space="PSUM") as psX,
        ):
            # ---------------- constants / staging ----------------
            ident = cpool.tile([128, 128], BF)
            nc.sync.dma_start(ident[:, :], id_d.ap())
            wq_sb = cpool.tile([128, HPC * D], BF, tag="wq")
            wq_sb2 = cpool.tile([128, HPC * D], BF, tag="wq2")
            nc.sync.dma_start(wq_sb[:, :], wq_d.ap()[0:128, :])
            nc.sync.dma_start(wq_sb2[:, :], wq_d.ap()[128:256, :])
            wkv_sb = cpool.tile([128, HPC * 2 * D], BF, tag="wkv")
            wkv_sb2 = cpool.tile([128, HPC * 2 * D], BF, tag="wkv2")
            nc.sync.dma_start(wkv_sb[:, :], wkv_d.ap()[0:128, :])
            nc.sync.dma_start(wkv_sb2[:, :], wkv_d.ap()[128:256, :])
            # Wout rows placed at partitions 0-31 / 64-95 to match the
            # oT_sb head layout (matmul requires equal base partitions)
            wout_sb = cpool.tile([128, C_S], BF, tag="wout")
            nc.sync.dma_start(wout_sb[0:D, :], wout_d.ap()[0:D, :])
            nc.sync.dma_start(wout_sb[64:64 + D, :], wout_d.ap()[D:2 * D, :])
            if use_g2:
                g2_sb = cpool.tile([128, HPC * D], BF, tag="g2")
                nc.sync.dma_start(g2_sb[:, :], g2_d.ap())

            s2T = [cpool.tile([128, N2], BF, tag=f"s2T{i}", name=f"s2T{i}")
                   for i in range(2)]
            s1T = [cpool.tile([128, N1], BF, tag=f"s1T{i}", name=f"s1T{i}")
                   for i in range(2)]
            for j in range(8):
                sl = slice(j * (N2 // 8), (j + 1) * (N2 // 8))
                for i in range(2):
                    nc.sync.dma_start(s2T[i][:, sl],
                                      s2T_d.ap()[i * 128:(i + 1) * 128, sl])
            for j in range(8):
                sl = slice(j * (N1 // 8), (j + 1) * (N1 // 8))
                for i in range(2):
                    nc.sync.dma_start(s1T[i][:, sl],
                                      s1T_d.ap()[i * 128:(i + 1) * 128, sl])

            kT = [cpool.tile([128, NKT], BF, tag=f"kT{h}", name=f"kT{h}")
                  for h in range(HPC)]
            qT = [cpool.tile([128, NQT], BF, tag=f"qT{h}", name=f"qT{h}")
                  for h in range(HPC)]
            # zero pad rows 33..127 (sectioned memsets so the first QK
            # doesn't wait on a whole-tensor op), mask-carrier row 32:
            # kT row32 = 0 on real chunks / kbnd on boundary; qT row32 = 1
            for h in range(HPC):
                for a in range(4):
                    ksl = slice(a * (NKT // 4), (a + 1) * (NKT // 4))
                    qsl = slice(a * (NQT // 4), (a + 1) * (NQT // 4))
                    nc.gpsimd.memset(kT[h][32:64, ksl], 0.0)
                    nc.gpsimd.memset(kT[h][64:128, ksl], 0.0)
                    nc.gpsimd.memset(qT[h][32:64, qsl], 0.0)
                    nc.gpsimd.memset(qT[h][64:128, qsl], 0.0)
                    nc.gpsimd.memset(qT[h][32:33, qsl], 1.0)
                if NB:
                    nc.sync.dma_start(kT[h][32:33, KCH * 128:NKT],
                                      kbnd_d.ap())

            vx = cpool.tile([128, NKC * HPC * VW], BF, tag="vx")
            nc.gpsimd.memset(
                vx[:, :].rearrange("p (n w) -> p n w", w=VW)[:, :, 32:33], 1.0)

            oT_sb = cpool.tile([128, NQT], BF, tag="oT")
            ss_all = cpool.tile([128, CQ * HPC + KCH * HPC + 2 * HPC], F32,
                                tag="ss")
            rinv_all = cpool.tile([128, CQ * HPC + KCH * HPC + 2 * HPC], F32,
                                  tag="rinv")

            # ---------------- norm pipeline ----------------
            # chunk ids: 0..KCH-1 kv, KCH..KCH+CQ-1 q
            def step1(ci, kc, j, kcp):
                """projection matmuls + psum->sbuf copies for one chunk.
                ci: global chunk id, kc: local chunk, j: index in batch,
                kcp: the batch's [128, 4*64] f32 staging tile."""
                kv = ci < KCH
                if kv:
                    sT, w1, w2, ncol = s2T, wkv_sb, wkv_sb2, HPC * 2 * D
                    src = kc * 128
                else:
                    sT, w1, w2, ncol = s1T, wq_sb, wq_sb2, HPC * D
                    src = qsrc(kc)
                pp = psX.tile([128, HPC * 2 * D], F32, tag="m",
                               name=f"pp{ci}")
                nc.tensor.matmul(pp[:, 0:ncol], sT[0][:, src:src + 128],
                                 w1[:, :], start=True, stop=False)
                nc.tensor.matmul(pp[:, 0:ncol], sT[1][:, src:src + 128],
                                 w2[:, :], start=False, stop=True)
                dst = kcp[:, j * HPC * D:(j + 1) * HPC * D]
                if kv:
                    # k cols [h,0:32]; v cols [h,32:64] -> vx section
                    nc.vector.tensor_copy(
                        dst.rearrange("p (h d) -> p h d", d=D),
                        pp[:, 0:ncol].rearrange("p (h x) -> p h x",
                                                h=HPC)[:, :, 0:D])
                    nc.vector.tensor_copy(
                        vx[:, kc * HPC * VW:(kc + 1) * HPC * VW]
                        .rearrange("p (h w) -> p h w", w=VW)[:, :, 0:D],
                        pp[:, 0:ncol].rearrange("p (h x) -> p h x",
                                                h=HPC)[:, :, D:2 * D])
                else:
                    nc.vector.tensor_copy(dst, pp[:, 0:ncol])

            def rsqrt_batch(sl, w, bid):
                # rinv = 1/sqrt(ss/D + eps) on GpSimd (bit-trick seed +
                # 2 Newton steps)
                xs = nrm.tile([128, 16], F32, tag="nx", name=f"nx{bid}")
                nc.gpsimd.tensor_scalar(xs[:, 0:w], ss_all[:, sl], 1.0 / D,
                                        EPS, AluOpType.mult, AluOpType.add)
                t = nrm.tile([128, 16], I32, tag="nt", name=f"nt{bid}")
                nc.vector.tensor_scalar(t[:, 0:w], xs[:, 0:w].bitcast(I32), 1,
                                        None, AluOpType.arith_shift_right)
                u = nrm.tile([128, 16], I32, tag="nu", name=f"nu{bid}")
                nc.vector.tensor_scalar(u[:, 0:w], t[:, 0:w], -1, 0x5F3759DF,
                                        AluOpType.mult, AluOpType.add)
                y = u[:, 0:w].bitcast(F32)
                for it in range(2):
                    a = nrm.tile([128, 16], F32, tag="na", name=f"na{bid}_{it}")
                    nc.gpsimd.tensor_tensor(a[:, 0:w], y, y, AluOpType.mult)
                    b = nrm.tile([128, 16], F32, tag="nb", name=f"nb{bid}_{it}")
                    nc.gpsimd.tensor_tensor(b[:, 0:w], a[:, 0:w], xs[:, 0:w],
                                            AluOpType.mult)
                    c = nrm.tile([128, 16], F32, tag="ncc",
                                 name=f"nc{bid}_{it}")
                    nc.gpsimd.tensor_scalar(c[:, 0:w], b[:, 0:w], -0.5, 1.5,
                                            AluOpType.mult, AluOpType.add)
                    dst = (nrm.tile([128, 16], F32, tag="ny",
                                    name=f"ny{bid}_{it}")
                           if it == 0 else None)
                    out = dst[:, 0:w] if it == 0 else rinv_all[:, sl]
                    nc.gpsimd.tensor_tensor(out, y, c[:, 0:w], AluOpType.mult)
                    y = out

            def finish(cis, kcs, kcp, bid):
                """square+reduce+rsqrt+scale+transpose+copy for a batch."""
                nb = len(cis)
                sq = sqp.tile([128, 4 * HPC * D], F32, tag="sq",
                              name=f"sq{bid}")
                nc.gpsimd.tensor_tensor(sq[:, 0:nb * HPC * D],
                                        kcp[:, 0:nb * HPC * D],
                                        kcp[:, 0:nb * HPC * D],
                                        AluOpType.mult)
                sl = slice(cis[0] * HPC, cis[0] * HPC + nb * HPC)
                nc.vector.reduce_sum(
                    ss_all[:, sl],
                    sq[:, 0:nb * HPC * D].rearrange("p (c d) -> p c d", d=D),
                    axis=AX.X)
                rsqrt_batch(sl, nb * HPC, bid)
                tp = psX.tile([HPC * D, 512], BF, tag="m", name=f"tp{bid}")
                for j, (ci, kc) in enumerate(zip(cis, kcs)):
                    pre = prep.tile([128, HPC * D], BF, tag="pre",
                                    name=f"pre{ci}")
                    for h in range(HPC):
                        nc.gpsimd.tensor_scalar(
                            pre[:, h * D:(h + 1) * D],
                            kcp[:, (j * HPC + h) * D:(j * HPC + h + 1) * D],
                            rinv_all[:, cis[0] * HPC + j * HPC + h:
                                     cis[0] * HPC + j * HPC + h + 1],
                            None, AluOpType.mult)
                    if use_g2 and ci >= KCH:
                        nc.gpsimd.tensor_tensor(pre[:, :], pre[:, :],
                                                g2_sb[:, :], AluOpType.mult)
                    nc.tensor.transpose(tp[:, j * 128:(j + 1) * 128],
                                        pre[:, :], ident[:, :])
                dstT = kT if cis[0] < KCH else qT
                dst0 = kcs[0] * 128
                for h in range(HPC):
                    nc.vector.tensor_copy(
                        dstT[h][0:D, dst0:dst0 + nb * 128],
                        tp[h * D:(h + 1) * D, 0:nb * 128])

            def boundary():
                for h in range(HPC):
                    nc.vector.tensor_copy(
                        kT[h][0:D, KCH * 128:KCH * 128 + NB * 128],
                        kT[h][0:D, BV * 128:BV * 128 + NB * 128])
                nc.vector.tensor_copy(
                    vx[:, KCH * HPC * VW:(KCH + NB) * HPC * VW],
                    vx[:, BV * HPC * VW:(BV + NB) * HPC * VW])

            # job queue: ordered closures with deadlines (bi, j) = must be
            # emitted before that attend iteration's QK
            jobs = []

            def norm_jobs(cis, kcs, deadline, bid):
                # the kcp staging tile is created lazily at first-pop so
                # pool-buffer rotation matches actual emission order
                box = {}

                def get_kcp():
                    if "t" not in box:
                        box["t"] = kcpp.tile([128, 4 * HPC * D], F32,
                                             tag="kcp", name=f"kcp{bid}")
                    return box["t"]

                for idx, (ci, kc) in enumerate(zip(cis, kcs)):
                    dl = (deadline[0], deadline[1] - (len(cis) - idx))
                    jobs.append((dl, lambda ci=ci, kc=kc, idx=idx:
                                 step1(ci, kc, idx, get_kcp())))
                jobs.append((deadline,
                             lambda: finish(cis, kcs, get_kcp(), bid)))

            def pop_jobs(now, budget=2):
                n = 0
                while jobs and (jobs[0][0] <= now or n < budget):
                    _, f = jobs.pop(0)
                    f()
                    n += 1

            # kv batches: batch k covers chunks 4k..4k+3, ready before
            # A-block0 iteration 4k (chunk 4k first used there)
            for k in range(1, KCH // 4):
                cs = list(range(4 * k, 4 * k + 4))
                norm_jobs(cs, cs, (0, 4 * k), f"kv{k}")
            if NB:
                # boundary copies need kv chunks BV..BV+NB-1 normed
                bnd_dl = (0, min(KCH - 1, 4 * ((BV + NB - 1) // 4) + 2))
                jobs.append((bnd_dl, boundary))
            # q batches: block bi needs q chunks for its vq range before
            # iteration 0 of that block
            qchunks = list(range(4, CQ))
            bi_of_chunk = {}
            for bi, (vq0, w, _, _) in enumerate(blocks):
                for c in range(vq0 // 128, (vq0 + w + 127) // 128):
                    bi_of_chunk.setdefault(c, bi)
            gi = 0
            while qchunks:
                grp = qchunks[:4]
                qchunks = qchunks[4:]
                need_bi = min(bi_of_chunk[c] for c in grp)
                norm_jobs([KCH + c for c in grp], grp, (need_bi, 0),
                          f"q{gi}")
                gi += 1

            # sort by deadline to get a sane emission order
            jobs.sort(key=lambda x: x[0])

            # prologue: kv chunks 0..3 + q chunks 0..3 (block 0 needs them)
            kcp0 = kcpp.tile([128, 4 * HPC * D], F32, tag="kcp", name="kcpP0")
            for j in range(4):
                step1(j, j, j, kcp0)
            finish([0, 1, 2, 3], [0, 1, 2, 3], kcp0, "kvP")
            kcp1 = kcpp.tile([128, 4 * HPC * D], F32, tag="kcp", name="kcpP1")
            for j in range(4):
                step1(KCH + j, j, j, kcp1)
            finish([KCH, KCH + 1, KCH + 2, KCH + 3], [0, 1, 2, 3], kcp1, "qP")

            # ---------------- attend + projection ----------------
            def proj(c):
                op0 = psX.tile([128, C_S], F32, tag="m", name=f"op0_{c}")
                nc.tensor.matmul(op0[:, :], oT_sb[0:D, c * 128:(c + 1) * 128],
                                 wout_sb[0:D, :], start=True, stop=True)
                os0 = osp.tile([128, C_S], F32, tag="os", name=f"os0_{c}")
                nc.scalar.activation(os0[:, :], op0[:, :], AF.Copy)
                nc.sync.dma_start(op0_d.ap()[c * 128:(c + 1) * 128, :],
                                  os0[:, :])
                op1 = psX.tile([128, C_S], F32, tag="m", name=f"op1_{c}")
                nc.tensor.matmul(op1[:, :],
                                 oT_sb[64:64 + D, c * 128:(c + 1) * 128],
                                 wout_sb[64:64 + D, :], start=True, stop=True)
                os1 = osp.tile([128, C_S], F32, tag="os", name=f"os1_{c}")
                nc.scalar.activation(os1[:, :], op1[:, :], AF.Copy)
                nc.sync.dma_start(op1_d.ap()[c * 128:(c + 1) * 128, :],
                                  os1[:, :])

            for bi, (vq0, w, kcs, tag) in enumerate(blocks):
                nkc = len(kcs)
                oT = psOT.tile([128, 512], F32, tag="oT", name=f"oT{bi}")
                scs = {}

                def qk(j, bi=bi, vq0=vq0, w=w, kcs=kcs, scs=scs):
                    kc = kcs[j]
                    sc = psSC.tile([128, 2 * 512], F32, tag="sc",
                                   name=f"sc{bi}_{j}")
                    scs[j] = sc
                    for h in range(HPC):
                        nc.tensor.matmul(
                            sc[:, h * w:(h + 1) * w],
                            kT[h][:, kc * 128:(kc + 1) * 128],
                            qT[h][:, vq0:vq0 + w], start=True, stop=True)

                for j in range(nkc):
                    pop_jobs((bi, j))
                    if j == 0:
                        qk(0)
                    kc = kcs[j]
                    sc = scs.pop(j)
                    ex = expp.tile([128, 2 * 512], BF, tag="ex",
                                   name=f"ex{bi}_{j}")
                    if exp_eng(bi, j, kc) == 'a':
                        nc.scalar.activation(ex[:, 0:2 * w], sc[:, 0:2 * w],
                                             AF.Exp, scale=SCALE)
                    else:
                        nc.vector.tensor_scalar(
                            ex[:, 0:2 * w].bitcast(I16), sc[:, 0:2 * w],
                            EXP_A, EXP_B, AluOpType.mult, AluOpType.add)
                    if j + 1 < nkc:
                        qk(j + 1)
                    for h in range(HPC):
                        nc.tensor.matmul(
                            oT[h * 64:h * 64 + VW, 0:w],
                            vx[:, (kc * HPC + h) * VW:(kc * HPC + h + 1) * VW],
                            ex[:, h * w:(h + 1) * w],
                            start=(j == 0), stop=(j == nkc - 1))
                # flush: [d | Z] rows -> SBUF bf16 (Z row 32 / 96 rides
                # along; host divides by Z)
                nc.vector.tensor_copy(oT_sb[0:VW, vq0:vq0 + w],
                                      oT[0:VW, 0:w])
                nc.vector.tensor_copy(oT_sb[64:64 + VW, vq0:vq0 + w],
                                      oT[64:64 + VW, 0:w])
                nc.sync.dma_start(z_d.ap()[0:1, vq0:vq0 + w],
                                  oT_sb[D:D + 1, vq0:vq0 + w])
                nc.sync.dma_start(z_d.ap()[1:2, vq0:vq0 + w],
                                  oT_sb[64 + D:64 + D + 1, vq0:vq0 + w])
                for c in range(vq0 // 128, (vq0 + w) // 128):
                    jobs.append(((len(blocks) + 1, 0),
                                 lambda c=c: proj(c)))

            while jobs:
                jobs.pop(0)[1]()

    nc.compile()
    return nc


def _host_prep(inputs):
    s1 = np.asarray(inputs["s1"], np.float32)
    s2 = np.asarray(inputs["s2"], np.float32)
    ridx1 = np.asarray(inputs["ridx1"], np.int32)
    ct1 = np.asarray(inputs["ct1"], np.int32)
    mask1 = np.asarray(inputs["mask1"], np.int32)
    mask2 = np.asarray(inputs["mask2"], np.int32)
    Wq = np.asarray(inputs["Wq"], np.float32)
    Wkv = np.asarray(inputs["Wkv"], np.float32)
    Wout = np.asarray(inputs["Wout"], np.float32)
    gq = np.asarray(inputs["gq"], np.float32)
    gk = np.asarray(inputs["gk"], np.float32)

    ct_idx = np.take_along_axis(ridx1, ct1[:, None], axis=1)
    pos = (ridx1 - ct_idx).astype(np.float32)
    half = C_S // 2
    freqs = np.exp(-np.log(10000.0) * np.arange(half, dtype=np.float32) / half)
    ang = pos[..., None] * freqs
    s1e = s1 + np.concatenate([np.sin(ang), np.cos(ang)],
                              axis=-1).astype(np.float32)

    g2 = gq * gk
    use_g2 = not np.allclose(g2, 1.0)

    perm_q = [np.argsort(mask1[b], kind="stable") for b in range(B)]
    perm_k = [np.argsort(1 - mask2[b], kind="stable") for b in range(B)]
    nqi = [int((mask1[b] == 0).sum()) for b in range(B)]
    nkv = [int((mask2[b] == 1).sum()) for b in range(B)]

    CAQ = (max(nqi) + 127) // 128
    BSTART = (min(nqi) // 128) * 128
    CBQ = (N1 - BSTART) // 128
    BV = min(nkv) // 128
    NB = (max(nkv) - BV * 128 + 127) // 128
    assert BV * 128 + NB * 128 >= max(nkv)
    cfg = (CAQ, CBQ, BSTART, BV, NB, use_g2)

    ident = np.eye(128, dtype=bf16)
    in_maps = []
    for c in range(NCORES):
        b, hp = c // 4, c % 4
        kb = np.zeros((1, max(1, NB) * 128), dtype=bf16)
        for jj in range(NB * 128):
            if BV * 128 + jj >= nkv[b]:
                kb[0, jj] = bf16(-INF / SCALE)
        m = {
            "s1T": np.ascontiguousarray(s1e[b][perm_q[b]].T).astype(bf16),
            "s2T": np.ascontiguousarray(s2[b][perm_k[b]].T).astype(bf16),
            "wq": np.ascontiguousarray(
                Wq[:, hp * HPC * D:(hp + 1) * HPC * D]).astype(bf16),
            "wkv": np.ascontiguousarray(
                Wkv[:, hp * HPC * 2 * D:(hp + 1) * HPC * 2 * D]).astype(bf16),
            "wout": np.ascontiguousarray(
                Wout[hp * HPC * D:(hp + 1) * HPC * D, :]).astype(bf16),
            "ident": ident,
            "kbnd": kb,
        }
        if use_g2:
            m["g2"] = np.tile(g2[None, hp * HPC * D:(hp + 1) * HPC * D],
                              (128, 1)).astype(bf16)
        in_maps.append(m)
    return in_maps, cfg, perm_q, nqi, np.asarray(inputs["b_out"], np.float32)


def _run(inputs, trace=False, **kw):
    in_maps, cfg, perm_q, nqi, b_out = _host_prep(inputs)
    CAQ, CBQ, BSTART = cfg[0], cfg[1], cfg[2]
    key = ("nc", cfg)
    if key not in _cache:
        _cache[key] = _build(cfg)
    nc = _cache[key]
    res = bass_utils.run_bass_kernel_spmd(
        nc, in_maps, core_ids=list(range(NCORES)), trace=trace, **kw)
    out = np.zeros((B, N1, C_S), np.float32)
    for c in range(NCORES):
        b = c // 4
        r = res.results[c]
        z = r["z"].astype(np.float32)
        oc = r["op0"] / np.maximum(z[0][:, None], 1e-30) \
            + r["op1"] / np.maximum(z[1][:, None], 1e-30)
        rows = np.empty((N1, C_S), np.float32)
        rows[:nqi[b]] = oc[:nqi[b]]
        vq = CAQ * 128 + (np.arange(nqi[b], N1) - BSTART)
        rows[nqi[b]:] = oc[vq]
        out[b][perm_q[b]] += rows
    out += b_out[None, None, :]
    return out, res


def kernel(**inputs) -> np.ndarray:
    out, _ = _run(inputs, trace=False)
    return out


# revision 18
# speedup vs baseline: 1.4463x; 1.4463x over previous
"""Trainium2 Bass kernel for nn_Attention_12146167513140.

Distributed dense attention over 8 NeuronCores; core c -> (batch c//4,
head-pair c%4).

Mask-sparsity restructure: softmax is shift-invariant, so mask1 only
matters through the product mask1*mask2 -- for an invalid (mask1=0) query
the row mask is constant and drops out, i.e. that row attends to ALL
keys unmasked; for a valid query only the valid (mask2=1) keys
contribute.  Host-side we permute queries invalid-first and keys
valid-first, then run two dense passes per core:
  pass A: q rows [0, CAQ*128)  x all 24 key chunks      (no mask)
  pass B: q rows [BSTART,3072) x BV full-valid chunks + NB boundary
          chunks (boundary = copy of chunks BV.. with an additive
          rank-1 mask row killing the invalid tail)
This cuts score/exp/PV work to ~75% and removes the mask row from the
main QK contraction.

Engine choreography per (block, key chunk): QK matmuls (PE) -> exp on
EITHER ScalarE (exact, fused *SCALE) or DVE (1-op Schraudolph: bf16
bit-pattern = int16(x*SCALE*128*log2e + (16256-C)); ~2% rel err) ->
PV matmuls accumulate [d|Z] x q in PSUM.  The softmax denominator Z
rides as a ones-column in PV; normalization happens on HOST after the
output projection (out = op0/Z0 + op1/Z1), so no on-device transposes
or reciprocals of Z are needed.  RMS-norm stats run on GpSimd (square/
rsqrt/scale) off a single batched PSUM->SBUF copy; projections + PE
transposes fill pipeline bubbles of the attend stream to keep the PE
p-state at max clock.
"""

import contextlib
import ctypes
import sys
import types

import numpy as np
import ml_dtypes

import concourse.bacc as bacc
import concourse.mybir as mybir
from concourse import bass_utils
from concourse.tile import TileContext
from concourse.alu_op_type import AluOpType
from concourse.mybir import ActivationFunctionType as AF


def _ensure_trace_support():
    """The container's antenv package lacks axon_hooks; bass_utils
    imports it when tracing is requested (e.g. via BASS_TRACE).  Install
    a functional shim so a traced run works instead of crashing, and
    make the artifact upload a no-op (no bucket access here)."""
    try:
        import antenv.axon_hooks  # noqa: F401
        return
    except ImportError:
        pass
    mod = types.ModuleType("antenv.axon_hooks")
    mod._hook = None
    mod.set_axon_ntff_profile_hook = lambda h: setattr(mod, "_hook", h)
    mod.get_axon_ntff_profile_hook = lambda: mod._hook
    try:
        import antenv
        sys.modules["antenv.axon_hooks"] = mod
        antenv.axon_hooks = mod
    except ImportError:
        sys.modules["antenv.axon_hooks"] = mod

    def _ntff_hook(so_path):
        try:
            lib = ctypes.CDLL(so_path)
        except OSError:
            return None
        if not hasattr(lib, "axon_start_nrt_profile"):
            return None
        lib.axon_start_nrt_profile.argtypes = [ctypes.POINTER(ctypes.c_int64),
                                               ctypes.c_size_t]
        lib.axon_start_nrt_profile.restype = ctypes.c_int64
        lib.axon_stop_nrt_profile.argtypes = [ctypes.c_char_p]
        lib.axon_stop_nrt_profile.restype = ctypes.c_int64

        @contextlib.contextmanager
        def _hook(output_dir, device_ids):
            import jax
            jax.devices()
            if device_ids:
                ids = (ctypes.c_int64 * len(device_ids))(*device_ids)
                rc = lib.axon_start_nrt_profile(ids, len(device_ids))
            else:
                rc = lib.axon_start_nrt_profile(None, 0)
            if rc != 0:
                raise RuntimeError(f"axon_start_nrt_profile rc={rc}")
            try:
                yield
            finally:
                lib.axon_stop_nrt_profile(str(output_dir).encode())

        return _hook

    mod.set_axon_ntff_profile_hook(_ntff_hook("/opt/axon/libaxon_pjrt.so"))

    _orig_upload = bass_utils.upload_artifacts

    def _safe_upload(tmpdir):
        try:
            return _orig_upload(tmpdir)
        except Exception:
            return tmpdir

    bass_utils.upload_artifacts = _safe_upload


_ensure_trace_support()

AX = mybir.AxisListType
I16 = mybir.dt.int16
I32 = mybir.dt.int32
BF = mybir.dt.bfloat16
F32 = mybir.dt.float32
bf16 = ml_dtypes.bfloat16

B, N1, N2 = 2, 3072, 3072
C_S, H, D = 256, 8, 32
INF = 100000.0
EPS = 1e-8
SCALE = float(np.sqrt(1.0 / (3 * D)))

NCORES = 8
HPC = 2              # heads per core
KCH = N2 // 128      # 24 real key chunks
VW = D + 1           # 33: v columns + ones column for Z

# Schraudolph exp in bf16 bit space: bits = x*SCALE*128*log2e + (16256 - C)
EXP_C = 5.77
EXP_A = SCALE * float(np.log2(np.e)) * 128.0
EXP_B = 127.0 * 128.0 - EXP_C

_cache = {}


def _build(cfg):
    CAQ, CBQ, BSTART, BV, NB, use_g2 = cfg
    CQ = CAQ + CBQ                 # total q chunks
    NQT = CQ * 128                 # q extent (gathered-virtual coordinate)
    NKC = KCH + NB                 # kv chunk sections incl boundary
    NKT = NKC * 128

    # q chunk -> source column in gathered s1T
    def qsrc(c):
        return c * 128 if c < CAQ else BSTART + (c - CAQ) * 128

    # attend blocks: (vq0, width, kc list, tag)
    blocks = []
    off, rem = 0, CAQ * 128
    while rem > 0:
        w = min(512, rem)
        blocks.append((off, w, list(range(KCH)), 'A'))
        off += w
        rem -= w
    bkcs = list(range(BV)) + [KCH + i for i in range(NB)]
    rem = CBQ * 128
    while rem > 0:
        w = min(512, rem)
        blocks.append((off, w, bkcs, 'B'))
        off += w
        rem -= w

    # exp engine: boundary chunks always ACT; elsewhere mix in DVE
    def exp_eng(bi, j, kc):
        if kc >= KCH:
            return 'a'
        if bi == 0:
            return 'a'
        return 'v' if j % 5 in (1, 3) else 'a'

    nc = bacc.Bacc("TRN2", target_bir_lowering=False, debug=False,
                   num_devices=NCORES)

    s1T_d = nc.dram_tensor("s1T", [C_S, N1], BF, kind="ExternalInput")
    s2T_d = nc.dram_tensor("s2T", [C_S, N2], BF, kind="ExternalInput")
    wq_d = nc.dram_tensor("wq", [C_S, HPC * D], BF, kind="ExternalInput")
    wkv_d = nc.dram_tensor("wkv", [C_S, HPC * 2 * D], BF, kind="ExternalInput")
    wout_d = nc.dram_tensor("wout", [HPC * D, C_S], BF, kind="ExternalInput")
    id_d = nc.dram_tensor("ident", [128, 128], BF, kind="ExternalInput")
    kbnd_d = nc.dram_tensor("kbnd", [1, max(1, NB) * 128], BF,
                            kind="ExternalInput")
    if use_g2:
        g2_d = nc.dram_tensor("g2", [128, HPC * D], BF, kind="ExternalInput")
    op0_d = nc.dram_tensor("op0", [NQT, C_S], F32, kind="ExternalOutput")
    op1_d = nc.dram_tensor("op1", [NQT, C_S], F32, kind="ExternalOutput")
    z_d = nc.dram_tensor("z", [HPC, NQT], BF, kind="ExternalOutput")

    with TileContext(nc) as tc:
        with (
            tc.tile_pool(name="const", bufs=1) as cpool,
            tc.tile_pool(name="kcpp", bufs=3) as kcpp,
            tc.tile_pool(name="sqp", bufs=3) as sqp,
            tc.tile_pool(name="prep", bufs=6) as prep,
            tc.tile_pool(name="nrm", bufs=4) as nrm,
            tc.tile_pool(name="expp", bufs=5) as expp,
            tc.tile_pool(name="osp", bufs=4) as osp,
            tc.tile_pool(name="psSC", bufs=2, space="PSUM") as psSC,
            tc.tile_pool(name="psOT", bufs=2, space="PSUM") as psOT,
            tc.tile_pool(name="psX", bufs=2, space="PSUM") as psX,
        ):
            # ---------------- constants / staging ----------------
            ident = cpool.tile([128, 128], BF)
            nc.sync.dma_start(ident[:, :], id_d.ap())
            wq_sb = cpool.tile([128, HPC * D], BF, tag="wq")
            wq_sb2 = cpool.tile([128, HPC * D], BF, tag="wq2")
            nc.sync.dma_start(wq_sb[:, :], wq_d.ap()[0:128, :])
            nc.sync.dma_start(wq_sb2[:, :], wq_d.ap()[128:256, :])
            wkv_sb = cpool.tile([128, HPC * 2 * D], BF, tag="wkv")
            wkv_sb2 = cpool.tile([128, HPC * 2 * D], BF, tag="wkv2")
            nc.sync.dma_start(wkv_sb[:, :], wkv_d.ap()[0:128, :])
            nc.sync.dma_start(wkv_sb2[:, :], wkv_d.ap()[128:256, :])
            # Wout rows placed at partitions 0-31 / 64-95 to match the
            # oT_sb head layout (matmul requires equal base partitions)
            wout_sb = cpool.tile([128, C_S], BF, tag="wout")
            nc.sync.dma_start(wout_sb[0:D, :], wout_d.ap()[0:D, :])
            nc.sync.dma_start(wout_sb[64:64 + D, :], wout_d.ap()[D:2 * D, :])
            if use_g2:
                g2_sb = cpool.tile([128, HPC * D], BF, tag="g2")
                nc.sync.dma_start(g2_sb[:, :], g2_d.ap())

            s2T = [cpool.tile([128, N2], BF, tag=f"s2T{i}", name=f"s2T{i}")
                   for i in range(2)]
            s1T = [cpool.tile([128, N1], BF, tag=f"s1T{i}", name=f"s1T{i}")
                   for i in range(2)]
            for j in range(8):
                sl = slice(j * (N2 // 8), (j + 1) * (N2 // 8))
                for i in range(2):
                    nc.sync.dma_start(s2T[i][:, sl],
                                      s2T_d.ap()[i * 128:(i + 1) * 128, sl])
            for j in range(8):
                sl = slice(j * (N1 // 8), (j + 1) * (N1 // 8))
                for i in range(2):
                    nc.sync.dma_start(s1T[i][:, sl],
                                      s1T_d.ap()[i * 128:(i + 1) * 128, sl])

            kT = [cpool.tile([128, NKT], BF, tag=f"kT{h}", name=f"kT{h}")
                  for h in range(HPC)]
            qT = [cpool.tile([128, NQT], BF, tag=f"qT{h}", name=f"qT{h}")
                  for h in range(HPC)]
            # zero pad rows 33..127 (sectioned memsets so the first QK
            # doesn't wait on a whole-tensor op), mask-carrier row 32:
            # kT row32 = 0 on real chunks / kbnd on boundary; qT row32 = 1
            for a in range(4):
                for h in range(HPC):
                    ksl = slice(a * (NKT // 4), (a + 1) * (NKT // 4))
                    qsl = slice(a * (NQT // 4), (a + 1) * (NQT // 4))
                    nc.gpsimd.memset(kT[h][32:64, ksl], 0.0)
                    nc.gpsimd.memset(kT[h][64:128, ksl], 0.0)
                    nc.gpsimd.memset(qT[h][32:64, qsl], 0.0)
                    nc.gpsimd.memset(qT[h][64:128, qsl], 0.0)
                    nc.gpsimd.memset(qT[h][32:33, qsl], 1.0)
            if NB:
                for h in range(HPC):
                    nc.sync.dma_start(kT[h][32:33, KCH * 128:NKT],
                                      kbnd_d.ap())

            vx = cpool.tile([128, NKC * HPC * VW], BF, tag="vx")
            nc.gpsimd.memset(
                vx[:, :].rearrange("p (n w) -> p n w", w=VW)[:, :, 32:33], 1.0)

            oT_sb = cpool.tile([128, NQT], BF, tag="oT")
            ss_all = cpool.tile([128, CQ * HPC + KCH * HPC + 2 * HPC], F32,
                                tag="ss")
            rinv_all = cpool.tile([128, CQ * HPC + KCH * HPC + 2 * HPC], F32,
                                  tag="rinv")

            # ---------------- norm pipeline ----------------
            # chunk ids: 0..KCH-1 kv, KCH..KCH+CQ-1 q
            def step1(ci, kc, j, kcp):
                """projection matmuls + psum->sbuf copies for one chunk.
                ci: global chunk id, kc: local chunk, j: index in batch,
                kcp: the batch's [128, 4*64] f32 staging tile."""
                kv = ci < KCH
                if kv:
                    sT, w1, w2, ncol = s2T, wkv_sb, wkv_sb2, HPC * 2 * D
                    src = kc * 128
                else:
                    sT, w1, w2, ncol = s1T, wq_sb, wq_sb2, HPC * D
                    src = qsrc(kc)
                pp = psX.tile([128, HPC * 2 * D], F32, tag="m",
                               name=f"pp{ci}")
                nc.tensor.matmul(pp[:, 0:ncol], sT[0][:, src:src + 128],
                                 w1[:, :], start=True, stop=False)
                nc.tensor.matmul(pp[:, 0:ncol], sT[1][:, src:src + 128],
                                 w2[:, :], start=False, stop=True)
                dst = kcp[:, j * HPC * D:(j + 1) * HPC * D]
                if kv:
                    # k cols [h,0:32]; v cols [h,32:64] -> vx section
                    nc.vector.tensor_copy(
                        dst.rearrange("p (h d) -> p h d", d=D),
                        pp[:, 0:ncol].rearrange("p (h x) -> p h x",
                                                h=HPC)[:, :, 0:D])
                    nc.vector.tensor_copy(
                        vx[:, kc * HPC * VW:(kc + 1) * HPC * VW]
                        .rearrange("p (h w) -> p h w", w=VW)[:, :, 0:D],
                        pp[:, 0:ncol].rearrange("p (h x) -> p h x",
                                                h=HPC)[:, :, D:2 * D])
                else:
                    nc.vector.tensor_copy(dst, pp[:, 0:ncol])

            def rsqrt_batch(sl, w, bid):
                # rinv = 1/sqrt(ss/D + eps) on GpSimd (bit-trick seed +
                # 2 Newton steps)
                xs = nrm.tile([128, 16], F32, tag="nx", name=f"nx{bid}")
                nc.vector.tensor_scalar(xs[:, 0:w], ss_all[:, sl], 1.0 / D,
                                        EPS, AluOpType.mult, AluOpType.add)
                t = nrm.tile([128, 16], I32, tag="nt", name=f"nt{bid}")
                nc.vector.tensor_scalar(t[:, 0:w], xs[:, 0:w].bitcast(I32), 1,
                                        None, AluOpType.arith_shift_right)
                u = nrm.tile([128, 16], I32, tag="nu", name=f"nu{bid}")
                nc.vector.tensor_scalar(u[:, 0:w], t[:, 0:w], -1, 0x5F3759DF,
                                        AluOpType.mult, AluOpType.add)
                y = u[:, 0:w].bitcast(F32)
                for it in range(2):
                    a = nrm.tile([128, 16], F32, tag="na", name=f"na{bid}_{it}")
                    nc.vector.tensor_tensor(a[:, 0:w], y, y, AluOpType.mult)
                    b = nrm.tile([128, 16], F32, tag="nb", name=f"nb{bid}_{it}")
                    nc.vector.tensor_tensor(b[:, 0:w], a[:, 0:w], xs[:, 0:w],
                                            AluOpType.mult)
                    c = nrm.tile([128, 16], F32, tag="ncc",
                                 name=f"nc{bid}_{it}")
                    nc.vector.tensor_scalar(c[:, 0:w], b[:, 0:w], -0.5, 1.5,
                                            AluOpType.mult, AluOpType.add)
                    dst = (nrm.tile([128, 16], F32, tag="ny",
                                    name=f"ny{bid}_{it}")
                           if it == 0 else None)
                    out = dst[:, 0:w] if it == 0 else rinv_all[:, sl]
                    nc.vector.tensor_tensor(out, y, c[:, 0:w], AluOpType.mult)
                    y = out

            def finish(cis, kcs, kcp, bid):
                """square+reduce+rsqrt+scale+transpose+copy for a batch."""
                nb = len(cis)
                sq = sqp.tile([128, 4 * HPC * D], F32, tag="sq",
                              name=f"sq{bid}")
                nc.vector.tensor_tensor(sq[:, 0:nb * HPC * D],
                                        kcp[:, 0:nb * HPC * D],
                                        kcp[:, 0:nb * HPC * D],
                                        AluOpType.mult)
                sl = slice(cis[0] * HPC, cis[0] * HPC + nb * HPC)
                nc.vector.reduce_sum(
                    ss_all[:, sl],
                    sq[:, 0:nb * HPC * D].rearrange("p (c d) -> p c d", d=D),
                    axis=AX.X)
                rsqrt_batch(sl, nb * HPC, bid)
                # scale all nb*HPC (chunk, head) groups in ONE DVE op via a
                # stride-0 broadcast of rinv over the D columns
                preb = prep.tile([128, 4 * HPC * D], BF, tag="pre",
                                 name=f"pre{bid}")
                rb = rinv_all[:, sl]
                rbb = type(rb)(rb.tensor, rb.offset, rb.ap + [[0, D]])
                nc.vector.tensor_tensor(
                    preb[:, 0:nb * HPC * D].rearrange("p (c d) -> p c d", d=D),
                    kcp[:, 0:nb * HPC * D].rearrange("p (c d) -> p c d", d=D),
                    rbb, AluOpType.mult)
                if use_g2 and cis[0] >= KCH:
                    g2b = g2_sb[:, :]
                    g2bb = type(g2b)(g2b.tensor, g2b.offset,
                                     [g2b.ap[0], [0, nb]] + g2b.ap[1:])
                    nc.vector.tensor_tensor(
                        preb[:, 0:nb * HPC * D]
                        .rearrange("p (c x) -> p c x", c=nb),
                        preb[:, 0:nb * HPC * D]
                        .rearrange("p (c x) -> p c x", c=nb),
                        g2bb, AluOpType.mult)
                tp = psX.tile([HPC * D, 512], BF, tag="m", name=f"tp{bid}")
                for j, (ci, kc) in enumerate(zip(cis, kcs)):
                    nc.tensor.transpose(
                        tp[:, j * 128:(j + 1) * 128],
                        preb[:, j * HPC * D:(j + 1) * HPC * D],
                        ident[:, :])
                dstT = kT if cis[0] < KCH else qT
                dst0 = kcs[0] * 128
                for h in range(HPC):
                    nc.vector.tensor_copy(
                        dstT[h][0:D, dst0:dst0 + nb * 128],
                        tp[h * D:(h + 1) * D, 0:nb * 128])

            def boundary():
                for h in range(HPC):
                    nc.vector.tensor_copy(
                        kT[h][0:D, KCH * 128:KCH * 128 + NB * 128],
                        kT[h][0:D, BV * 128:BV * 128 + NB * 128])
                nc.vector.tensor_copy(
                    vx[:, KCH * HPC * VW:(KCH + NB) * HPC * VW],
                    vx[:, BV * HPC * VW:(BV + NB) * HPC * VW])

            # job queue: ordered closures with deadlines (bi, j) = must be
            # emitted before that attend iteration's QK
            jobs = []

            def norm_jobs(cis, kcs, deadline, bid):
                # the kcp staging tile is created lazily at first-pop so
                # pool-buffer rotation matches actual emission order
                box = {}

                def get_kcp():
                    if "t" not in box:
                        box["t"] = kcpp.tile([128, 4 * HPC * D], F32,
                                             tag="kcp", name=f"kcp{bid}")
                    return box["t"]

                for idx, (ci, kc) in enumerate(zip(cis, kcs)):
                    dl = (deadline[0], deadline[1] - (len(cis) - idx))
                    jobs.append((dl, lambda ci=ci, kc=kc, idx=idx:
                                 step1(ci, kc, idx, get_kcp())))
                jobs.append((deadline,
                             lambda: finish(cis, kcs, get_kcp(), bid)))

            def pop_jobs(now, budget=3):
                n = 0
                while jobs and (jobs[0][0] <= now or n < budget):
                    _, f = jobs.pop(0)
                    f()
                    n += 1

            # kv batches: batch k covers chunks 4k..4k+3, ready before
            # A-block0 iteration 4k (chunk 4k first used there)
            for k in range(1, KCH // 4):
                cs = list(range(4 * k, 4 * k + 4))
                norm_jobs(cs, cs, (0, 4 * k), f"kv{k}")
            if NB:
                # boundary copies need kv chunks BV..BV+NB-1 normed
                bnd_dl = (0, min(KCH - 1, 4 * ((BV + NB - 1) // 4) + 2))
                jobs.append((bnd_dl, boundary))
            # q batches: block bi needs q chunks for its vq range before
            # iteration 0 of that block
            qchunks = list(range(4, CQ))
            bi_of_chunk = {}
            for bi, (vq0, w, _, _) in enumerate(blocks):
                for c in range(vq0 // 128, (vq0 + w + 127) // 128):
                    bi_of_chunk.setdefault(c, bi)
            gi = 0
            while qchunks:
                grp = qchunks[:4]
                qchunks = qchunks[4:]
                need_bi = min(bi_of_chunk[c] for c in grp)
                dl = (need_bi - 1, 6) if need_bi > 0 else (0, 0)
                norm_jobs([KCH + c for c in grp], grp, dl, f"q{gi}")
                gi += 1

            # sort by deadline to get a sane emission order
            jobs.sort(key=lambda x: x[0])

            # prologue: kv chunks 0..3 + q chunks 0..3 (block 0 needs them)
            kcp0 = kcpp.tile([128, 4 * HPC * D], F32, tag="kcp", name="kcpP0")
            for j in range(4):
                step1(j, j, j, kcp0)
            finish([0, 1, 2, 3], [0, 1, 2, 3], kcp0, "kvP")
            kcp1 = kcpp.tile([128, 4 * HPC * D], F32, tag="kcp", name="kcpP1")
            for j in range(4):
                step1(KCH + j, j, j, kcp1)
            finish([KCH, KCH + 1, KCH + 2, KCH + 3], [0, 1, 2, 3], kcp1, "qP")

            # ---------------- attend + projection ----------------
            def proj(c):
                op0 = psX.tile([128, C_S], F32, tag="m", name=f"op0_{c}")
                nc.tensor.matmul(op0[:, :], oT_sb[0:D, c * 128:(c + 1) * 128],
                                 wout_sb[0:D, :], start=True, stop=True)
                os0 = osp.tile([128, C_S], F32, tag="os", name=f"os0_{c}")
                nc.scalar.activation(os0[:, :], op0[:, :], AF.Copy)
                nc.sync.dma_start(op0_d.ap()[c * 128:(c + 1) * 128, :],
                                  os0[:, :])
                op1 = psX.tile([128, C_S], F32, tag="m", name=f"op1_{c}")
                nc.tensor.matmul(op1[:, :],
                                 oT_sb[64:64 + D, c * 128:(c + 1) * 128],
                                 wout_sb[64:64 + D, :], start=True, stop=True)
                os1 = osp.tile([128, C_S], F32, tag="os", name=f"os1_{c}")
                nc.scalar.activation(os1[:, :], op1[:, :], AF.Copy)
                nc.sync.dma_start(op1_d.ap()[c * 128:(c + 1) * 128, :],
                                  os1[:, :])

            for bi, (vq0, w, kcs, tag) in enumerate(blocks):
                nkc = len(kcs)
                oT = psOT.tile([128, 512], F32, tag="oT", name=f"oT{bi}")
                scs = {}

                def qk(j, bi=bi, vq0=vq0, w=w, kcs=kcs, scs=scs):
                    kc = kcs[j]
                    sc = psSC.tile([128, 2 * 512], F32, tag="sc",
                                   name=f"sc{bi}_{j}")
                    scs[j] = sc
                    for h in range(HPC):
                        nc.tensor.matmul(
                            sc[:, h * w:(h + 1) * w],
                            kT[h][:, kc * 128:(kc + 1) * 128],
                            qT[h][:, vq0:vq0 + w], start=True, stop=True)

                for j in range(nkc):
                    pop_jobs((bi, j))
                    if j == 0:
                        qk(0)
                    kc = kcs[j]
                    sc = scs.pop(j)
                    ex = expp.tile([128, 2 * 512], BF, tag="ex",
                                   name=f"ex{bi}_{j}")
                    if exp_eng(bi, j, kc) == 'a':
                        nc.scalar.activation(ex[:, 0:2 * w], sc[:, 0:2 * w],
                                             AF.Exp, scale=SCALE)
                    else:
                        nc.vector.tensor_scalar(
                            ex[:, 0:2 * w].bitcast(I16), sc[:, 0:2 * w],
                            EXP_A, EXP_B, AluOpType.mult, AluOpType.add)
                    if j + 1 < nkc:
                        qk(j + 1)
                    for h in range(HPC):
                        nc.tensor.matmul(
                            oT[h * 64:h * 64 + VW, 0:w],
                            vx[:, (kc * HPC + h) * VW:(kc * HPC + h + 1) * VW],
                            ex[:, h * w:(h + 1) * w],
                            start=(j == 0), stop=(j == nkc - 1))
                # flush: [d | Z] rows -> SBUF bf16 (Z row 32 / 96 rides
                # along; host divides by Z)
                nc.vector.tensor_copy(oT_sb[0:VW, vq0:vq0 + w],
                                      oT[0:VW, 0:w])
                nc.vector.tensor_copy(oT_sb[64:64 + VW, vq0:vq0 + w],
                                      oT[64:64 + VW, 0:w])
                nc.sync.dma_start(z_d.ap()[0:1, vq0:vq0 + w],
                                  oT_sb[D:D + 1, vq0:vq0 + w])
                nc.sync.dma_start(z_d.ap()[1:2, vq0:vq0 + w],
                                  oT_sb[64 + D:64 + D + 1, vq0:vq0 + w])
                for c in range(vq0 // 128, (vq0 + w) // 128):
                    jobs.append(((len(blocks) + 1, 0),
                                 lambda c=c: proj(c)))

            while jobs:
                jobs.pop(0)[1]()

    nc.compile()
    return nc


def _host_prep(inputs):
    s1 = np.asarray(inputs["s1"], np.float32)
    s2 = np.asarray(inputs["s2"], np.float32)
    ridx1 = np.asarray(inputs["ridx1"], np.int32)
    ct1 = np.asarray(inputs["ct1"], np.int32)
    mask1 = np.asarray(inputs["mask1"], np.int32)
    mask2 = np.asarray(inputs["mask2"], np.int32)
    Wq = np.asarray(inputs["Wq"], np.float32)
    Wkv = np.asarray(inputs["Wkv"], np.float32)
    Wout = np.asarray(inputs["Wout"], np.float32)
    gq = np.asarray(inputs["gq"], np.float32)
    gk = np.asarray(inputs["gk"], np.float32)

    ct_idx = np.take_along_axis(ridx1, ct1[:, None], axis=1)
    pos = (ridx1 - ct_idx).astype(np.float32)
    half = C_S // 2
    freqs = np.exp(-np.log(10000.0) * np.arange(half, dtype=np.float32) / half)
    ang = pos[..., None] * freqs
    s1e = s1 + np.concatenate([np.sin(ang), np.cos(ang)],
                              axis=-1).astype(np.float32)

    g2 = gq * gk
    use_g2 = not np.allclose(g2, 1.0)

    perm_q = [np.argsort(mask1[b], kind="stable") for b in range(B)]
    perm_k = [np.argsort(1 - mask2[b], kind="stable") for b in range(B)]
    nqi = [int((mask1[b] == 0).sum()) for b in range(B)]
    nkv = [int((mask2[b] == 1).sum()) for b in range(B)]

    CAQ = (max(nqi) + 127) // 128
    BSTART = (min(nqi) // 128) * 128
    CBQ = (N1 - BSTART) // 128
    BV = min(nkv) // 128
    NB = (max(nkv) - BV * 128 + 127) // 128
    assert BV * 128 + NB * 128 >= max(nkv)
    cfg = (CAQ, CBQ, BSTART, BV, NB, use_g2)

    ident = np.eye(128, dtype=bf16)
    in_maps = []
    for c in range(NCORES):
        b, hp = c // 4, c % 4
        kb = np.zeros((1, max(1, NB) * 128), dtype=bf16)
        for jj in range(NB * 128):
            if BV * 128 + jj >= nkv[b]:
                kb[0, jj] = bf16(-INF / SCALE)
        m = {
            "s1T": np.ascontiguousarray(s1e[b][perm_q[b]].T).astype(bf16),
            "s2T": np.ascontiguousarray(s2[b][perm_k[b]].T).astype(bf16),
            "wq": np.ascontiguousarray(
                Wq[:, hp * HPC * D:(hp + 1) * HPC * D]).astype(bf16),
            "wkv": np.ascontiguousarray(
                Wkv[:, hp * HPC * 2 * D:(hp + 1) * HPC * 2 * D]).astype(bf16),
            "wout": np.ascontiguousarray(
                Wout[hp * HPC * D:(hp + 1) * HPC * D, :]).astype(bf16),
            "ident": ident,
            "kbnd": kb,
        }
        if use_g2:
            m["g2"] = np.tile(g2[None, hp * HPC * D:(hp + 1) * HPC * D],
                              (128, 1)).astype(bf16)
        in_maps.append(m)
    return in_maps, cfg, perm_q, nqi, np.asarray(inputs["b_out"], np.float32)


def _run(inputs, trace=False, **kw):
    in_maps, cfg, perm_q, nqi, b_out = _host_prep(inputs)
    CAQ, CBQ, BSTART = cfg[0], cfg[1], cfg[2]
    key = ("nc", cfg)
    if key not in _cache:
        _cache[key] = _build(cfg)
    nc = _cache[key]
    res = bass_utils.run_bass_kernel_spmd(
        nc, in_maps, core_ids=list(range(NCORES)), trace=trace, **kw)
    out = np.zeros((B, N1, C_S), np.float32)
    for c in range(NCORES):
        b = c // 4
        r = res.results[c]
        z = r["z"].astype(np.float32)
        oc = r["op0"] / np.maximum(z[0][:, None], 1e-30) \
            + r["op1"] / np.maximum(z[1][:, None], 1e-30)
        rows = np.empty((N1, C_S), np.float32)
        rows[:nqi[b]] = oc[:nqi[b]]
        vq = CAQ * 128 + (np.arange(nqi[b], N1) - BSTART)
        rows[nqi[b]:] = oc[vq]
        out[b][perm_q[b]] += rows
    out += b_out[None, None, :]
    return out, res


def kernel(**inputs) -> np.ndarray:
    out, _ = _run(inputs, trace=False)
    return out
